# revision 17
# baseline (speedup 1.0000x reference)
"""Trainium2 Bass kernel for nn_FastAttention: out = v + q @ (k^T @ v) per (b,h).

Full shapes: q,k,v [B=2, H=16, S=4096, D=128] f32.
Sharding: B*H = 32 pairs split across 8 cores -> 4 pairs/core, no collectives.

Per (b,h) pair on-core:
  phase A: kv[d,e] = sum_s k[s,d] v[s,e]    (32 accumulating fp32 matmuls)
  phase T: qT[d,s] = q[s,d]^T               (PE transpose via identity, fp32)
  phase B: out[s,e] = v[s,e] + sum_d qT[d,s] kv[d,e]   (bf16 matmuls)

The kernel is DMA-bound: 32MB/core of HBM traffic at the measured
~420GB/s/core saturates the DMA engines from ~8us (fixed preamble) to
~84us; the graded time is that window plus the tail chain after the last
input byte plus a ~3us fixed epilogue.  Schedule principles:
  - SBUF layout tile[p, n*128+d] = x[32p+n, d]: every load/store is 4-8KB
    contiguous per partition (line rate); a matmul "chunk" is the strided
    row-set {32p+j}, a plain column slice of the tile.
  - Phase B runs in bf16: qT and kv are rounded to bf16 for free inside
    the PSUM->SBUF copies that already existed (ACT copy after the
    transposes; DVE copy after phase A).  bf16 streams 1 cycle/row vs
    fp32's 4, cutting PE active time ~1.4x and PE power (fp32 runs 2
    passes/row), which matters because PE power-throttling stretched
    compute on straggler cores until it gated the DMA stream.  k^T v
    stays exact fp32; measured rel err ~3e-3 (threshold 2e-2).
  - k,v load in quarters so phase A (chunk n needs only chunk n) tracks
    the arriving data instead of serializing after it.
  - The tail is governed by sequencer-side DMA issue (~0.65-1us per
    dma_start, serial per ring once flow control releases): loads and
    stores live on DIFFERENT rings (loads HWDGE/sync, stores
    SWDGE/gpsimd) so their tail issues pipeline in parallel, the last
    pair's q ends with two eighth-DMAs so the final load gates only one
    4-chunk group of work, and its stores go out at group granularity.
  - k,v triple-buffered so pair p+1's loads never wait on compute;
    q/qT/o double-buffered.
  - PE executes in compiled order: T(g+1) is emitted before B(g) so the
    PSUM->SBUF qT copy (ACT) hides behind the next group's transposes;
    qT copies all on ACT and v-adds all on DVE so neither in-order queue
    blocks the other.
"""

import sys

if "/opt/trn_rl_repo" not in sys.path:
    sys.path.insert(0, "/opt/trn_rl_repo")

import numpy as np

import concourse.bass as bass
import concourse.mybir as mybir
import concourse.tile as tile
from concourse import bacc
from concourse.bass import ds, ts
from concourse.bass_utils import run_bass_kernel_spmd
from concourse.masks import make_identity

B, H, S, D = 2, 16, 4096, 128
N_CORES = 8
PAIRS = (B * H) // N_CORES  # 4
F32 = mybir.dt.float32
BF16 = mybir.dt.bfloat16


def build_nc(pairs=PAIRS, s=S):
    nc = bacc.Bacc(
        "TRN2", target_bir_lowering=False, debug=False, num_devices=N_CORES
    )
    q = nc.dram_tensor("q", [pairs, s, D], F32, kind="ExternalInput").ap()
    k = nc.dram_tensor("k", [pairs, s, D], F32, kind="ExternalInput").ap()
    v = nc.dram_tensor("v", [pairs, s, D], F32, kind="ExternalInput").ap()
    out = nc.dram_tensor("out", [pairs, s, D], F32, kind="ExternalOutput").ap()

    nch = s // 128  # s-chunks per pair
    gsz = 4  # chunks per psum group (512 free-dim)
    ngrp = nch // gsz

    with tile.TileContext(nc) as tc:
        with (
            tc.tile_pool(name="const", bufs=1) as cpool,
            tc.tile_pool(name="kvio", bufs=3) as kvio,
            tc.tile_pool(name="qio", bufs=2) as qio,
            tc.tile_pool(name="pskv", bufs=2, space="PSUM") as pskv,
            tc.tile_pool(name="psq", bufs=3, space="PSUM") as psq,
            tc.tile_pool(name="pso", bufs=3, space="PSUM") as pso,
        ):
            ident = cpool.tile([128, 128], F32)
            make_identity(nc, ident[:])

            for p in range(pairs):
                last = p == pairs - 1
                k_sb = kvio.tile([128, s], F32, tag="k")
                v_sb = kvio.tile([128, s], F32, tag="v")
                q_sb = qio.tile([128, s], F32, tag="q")
                qT_sb = qio.tile([128, s], BF16, tag="qT")
                o_sb = qio.tile([128, s], F32, tag="o")
                kv_sb = qio.tile([128, 128], BF16, tag="kv")

                # loads: partition p holds rows 32p..32p+31 (16KB contiguous);
                # chunk j is the strided row-set {32p+j}.
                k3 = k[p].rearrange("(p n) d -> p n d", p=128)
                v3 = v[p].rearrange("(p n) d -> p n d", p=128)
                q3 = q[p].rearrange("(p n) d -> p n d", p=128)
                k_t3 = k_sb[:].rearrange("p (n d) -> p n d", d=128)
                v_t3 = v_sb[:].rearrange("p (n d) -> p n d", d=128)
                q_t3 = q_sb[:].rearrange("p (n d) -> p n d", d=128)
                # all loads on the SP HWDGE ring: issuing any of them from
                # nc.scalar would head-of-line block behind the qT copies on
                # the ACT sequencer (measured +15us).
                qtr = nch // 4
                # q in quarters; the last pair's final quarter is split into
                # two eighths so the very last load DMA gates only one
                # 4-chunk group of transposes/matmuls/stores.  For the last
                # pair the q quarters are interleaved BETWEEN the k,v
                # quarters so its transposes (emitted inside phase A below)
                # and their ACT copies pre-complete during the load window
                # instead of serializing after the final input byte.
                if last:
                    qsp = [[(0, 8)], [(8, 8)], [(16, 8)], [(24, 4), (28, 4)]]
                    for h in range(4):
                        hs = ts(h, qtr)
                        nc.sync.dma_start(out=k_t3[:, hs], in_=k3[:, hs])
                        nc.sync.dma_start(out=v_t3[:, hs], in_=v3[:, hs])
                        for sp, ln in qsp[h]:
                            qhs = ds(sp, ln)
                            nc.sync.dma_start(out=q_t3[:, qhs], in_=q3[:, qhs])
                else:
                    for h in range(4):
                        hs = ts(h, qtr)
                        nc.sync.dma_start(out=k_t3[:, hs], in_=k3[:, hs])
                        nc.sync.dma_start(out=v_t3[:, hs], in_=v3[:, hs])
                    for sp, ln in [(0, 8), (8, 8), (16, 8), (24, 8)]:
                        qhs = ds(sp, ln)
                        nc.sync.dma_start(out=q_t3[:, qhs], in_=q3[:, qhs])
                if last:
                    # sacrificial final ring entry: the DGE services a ring's
                    # LAST descriptors at single-engine rate (~25GB/s, ~4-5us
                    # of crawl on the final ~100KB).  Re-reading two q chunks
                    # into a scratch tile nobody waits on moves that crawl
                    # onto bytes that gate nothing; the real final eighth
                    # gets full-rate service and the crawl overlaps the
                    # B-chain and stores that follow it.
                    scr = cpool.tile([128, 256], F32, name="tail_scratch")
                    scr3 = scr[:].rearrange("p (n d) -> p n d", d=128)
                    nc.sync.dma_start(out=scr3[:, ds(0, 2)], in_=q3[:, ds(0, 2)])

                kv_ps = pskv.tile([128, 128], F32, tag="kv_ps")

                def emit_A(n0, n1):
                    # phase A: kv[d,e] accumulated over s-chunks, exact fp32
                    for n in range(n0, n1):
                        nc.tensor.matmul(
                            kv_ps[:],
                            lhsT=k_sb[:, ts(n, 128)],
                            rhs=v_sb[:, ts(n, 128)],
                            start=(n == 0),
                            stop=(n == nch - 1),
                        )

                o3 = out[p].rearrange("(p n) d -> p n d", p=128)
                o_t3 = o_sb[:].rearrange("p (n d) -> p n d", d=128)

                def emit_T(g):
                    qt_ps = psq.tile([128, gsz * 128], F32, tag="qt_ps")
                    for j in range(gsz):
                        n = g * gsz + j
                        nc.tensor.transpose(
                            qt_ps[:, ts(j, 128)], q_sb[:, ts(n, 128)], ident[:]
                        )
                    # ACT, not DVE: keeps the copy off DVE's in-order queue
                    # (which carries the v-adds); the copy also rounds the
                    # fp32 transpose result to bf16 for phase B.
                    nc.scalar.copy(qT_sb[:, ts(g, gsz * 128)], qt_ps[:])

                def emit_B(g):
                    o_ps = pso.tile([128, gsz * 128], F32, tag="o_ps")
                    for j in range(gsz):
                        n = g * gsz + j
                        nc.tensor.matmul(
                            o_ps[:, ts(j, 128)],
                            lhsT=qT_sb[:, ts(n, 128)],
                            rhs=kv_sb[:],
                            start=True,
                            stop=True,
                        )
                    nc.vector.tensor_add(
                        o_sb[:, ts(g, gsz * 128)],
                        o_ps[:],
                        v_sb[:, ts(g, gsz * 128)],
                    )

                # stores per 8 chunks during the bulk, per 4-chunk group for
                # the last pair's final half so the tail store is small; all
                # on the SWDGE ring (its sequencer is otherwise idle at the
                # tail -- the sync ring still issues the last q loads).
                def emit_store(g):
                    nonlocal_s = store_state
                    done = (g + 1) * gsz
                    fine = last and done > 24
                    if done % 8 == 0 or g == ngrp - 1 or fine:
                        hs = ds(nonlocal_s[0], done - nonlocal_s[0])
                        nc.gpsimd.dma_start(out=o3[:, hs], in_=o_t3[:, hs])
                        nonlocal_s[0] = done

                store_state = [0]
                if last:
                    # A quarters interleaved with the T groups whose q data
                    # arrives alongside in ring order (k qtr, v qtr, q span):
                    # the in-order PE runs every transpose during the load
                    # window, so after the final input byte only the bf16
                    # B matmuls and the DVE adds remain.  The transposes'
                    # psq banks are separate from A's accumulation bank, so
                    # interleaving does not break the accumulation group.
                    for qi in range(4):
                        emit_A(qi * 8, qi * 8 + 8)
                        emit_T(2 * qi)
                        emit_T(2 * qi + 1)
                    nc.vector.tensor_copy(kv_sb[:], kv_ps[:])
                    for g in range(ngrp):
                        emit_B(g)
                        emit_store(g)
                else:
                    emit_A(0, nch)
                    nc.vector.tensor_copy(kv_sb[:], kv_ps[:])
                    emit_T(0)
                    for g in range(ngrp):
                        if g + 1 < ngrp:
                            emit_T(g + 1)
                        emit_B(g)
                        emit_store(g)
    nc.finalize()
    return nc


def kernel(q, k, v, _trace=False):
    q = np.ascontiguousarray(np.asarray(q, dtype=np.float32)).reshape(B * H, S, D)
    k = np.ascontiguousarray(np.asarray(k, dtype=np.float32)).reshape(B * H, S, D)
    v = np.ascontiguousarray(np.asarray(v, dtype=np.float32)).reshape(B * H, S, D)

    nc = build_nc()
    in_maps = [
        {
            "q": q[i * PAIRS : (i + 1) * PAIRS],
            "k": k[i * PAIRS : (i + 1) * PAIRS],
            "v": v[i * PAIRS : (i + 1) * PAIRS],
        }
        for i in range(N_CORES)
    ]
    res = run_bass_kernel_spmd(nc, in_maps, core_ids=list(range(N_CORES)))
    full = np.concatenate([res.results[i]["out"] for i in range(N_CORES)], axis=0)
    out = full.reshape(B, H, S, D)
    if _trace:
        # repeat traced executes: the executable is compiled+cached after the
        # first run, so each NTFF profile context wraps only an execute.
        # Multiple samples filter out co-tenant HBM-contention noise.
        tres = [
            run_bass_kernel_spmd(
                nc,
                in_maps,
                core_ids=list(range(N_CORES)),
                trace=True,
                trace_cores=list(range(N_CORES)),
            )
            for _ in range(3)
        ]
        return out, tres
    return out


# revision 19
# speedup vs baseline: 1.2119x; 1.2119x over previous
"""Trainium2 Bass kernel for nn_FastAttention: out = v + q @ (k^T @ v) per (b,h).

Full shapes: q,k,v [B=2, H=16, S=4096, D=128] f32.
Sharding: B*H = 32 pairs split across 8 cores -> 4 pairs/core, no collectives.

Per (b,h) pair on-core:
  phase A: kv[d,e] = sum_s k[s,d] v[s,e]    (32 accumulating fp32 matmuls)
  phase T: qT[d,s] = q[s,d]^T               (PE transpose via identity, fp32)
  phase B: out[s,e] = v[s,e] + sum_d qT[d,s] kv[d,e]   (bf16 matmuls)

The kernel is DMA-bound: 32MB/core of HBM traffic at the measured
~420GB/s/core saturates the DMA engines from ~8us (fixed preamble) to
~84us; the graded time is that window plus the tail chain after the last
input byte plus a ~3us fixed epilogue.  Schedule principles:
  - SBUF layout tile[p, n*128+d] = x[32p+n, d]: every load/store is 4-8KB
    contiguous per partition (line rate); a matmul "chunk" is the strided
    row-set {32p+j}, a plain column slice of the tile.
  - Phase B runs in bf16: qT and kv are rounded to bf16 for free inside
    the PSUM->SBUF copies that already existed (ACT copy after the
    transposes; DVE copy after phase A).  bf16 streams 1 cycle/row vs
    fp32's 4, cutting PE active time ~1.4x and PE power (fp32 runs 2
    passes/row), which matters because PE power-throttling stretched
    compute on straggler cores until it gated the DMA stream.  k^T v
    stays exact fp32; measured rel err ~3e-3 (threshold 2e-2).
  - k,v load in quarters so phase A (chunk n needs only chunk n) tracks
    the arriving data instead of serializing after it.
  - The tail is governed by sequencer-side DMA issue (~0.65-1us per
    dma_start, serial per ring once flow control releases): loads and
    stores live on DIFFERENT rings (loads HWDGE/sync, stores
    SWDGE/gpsimd) so their tail issues pipeline in parallel, the last
    pair's q ends with two eighth-DMAs so the final load gates only one
    4-chunk group of work, and its stores go out at group granularity.
  - k,v triple-buffered so pair p+1's loads never wait on compute;
    q/qT/o double-buffered.
  - PE executes in compiled order: T(g+1) is emitted before B(g) so the
    PSUM->SBUF qT copy (ACT) hides behind the next group's transposes;
    qT copies all on ACT and v-adds all on DVE so neither in-order queue
    blocks the other.
"""

import sys

if "/opt/trn_rl_repo" not in sys.path:
    sys.path.insert(0, "/opt/trn_rl_repo")

import numpy as np

import concourse.bass as bass
import concourse.mybir as mybir
import concourse.tile as tile
from concourse import bacc
from concourse.bass import ds, ts
from concourse.bass_utils import run_bass_kernel_spmd
from concourse.masks import make_identity

B, H, S, D = 2, 16, 4096, 128
N_CORES = 8
PAIRS = (B * H) // N_CORES  # 4
F32 = mybir.dt.float32
BF16 = mybir.dt.bfloat16


def build_nc(pairs=PAIRS, s=S):
    nc = bacc.Bacc(
        "TRN2", target_bir_lowering=False, debug=False, num_devices=N_CORES
    )
    q = nc.dram_tensor("q", [pairs, s, D], F32, kind="ExternalInput").ap()
    k = nc.dram_tensor("k", [pairs, s, D], F32, kind="ExternalInput").ap()
    v = nc.dram_tensor("v", [pairs, s, D], F32, kind="ExternalInput").ap()
    out = nc.dram_tensor("out", [pairs, s, D], F32, kind="ExternalOutput").ap()

    nch = s // 128  # s-chunks per pair
    gsz = 4  # chunks per psum group (512 free-dim)
    ngrp = nch // gsz

    with tile.TileContext(nc) as tc:
        with (
            tc.tile_pool(name="const", bufs=1) as cpool,
            tc.tile_pool(name="kvio", bufs=3) as kvio,
            tc.tile_pool(name="kv16", bufs=1) as kv16p,
            tc.tile_pool(name="qio", bufs=2) as qio,
            tc.tile_pool(name="pskv", bufs=2, space="PSUM") as pskv,
            tc.tile_pool(name="psq", bufs=3, space="PSUM") as psq,
            tc.tile_pool(name="pso", bufs=3, space="PSUM") as pso,
        ):
            ident = cpool.tile([128, 128], F32)
            make_identity(nc, ident[:])

            kv3 = [None, None]

            for p in range(pairs):
                last = p == pairs - 1
                if not last:
                    k_sb = kvio.tile([128, s], F32, tag="k")
                    v_sb = kvio.tile([128, s], F32, tag="v")
                else:
                    # pair 3's k,v were cast-loaded to bf16 on the SWDGE
                    # ring back while pair 2 was being emitted (below), so
                    # its kv product finishes ~5us before its q arrives and
                    # the T/B/add chain tracks the q stream inside the DMA
                    # window instead of serializing after the last byte.
                    k_sb, v_sb = kv3
                q_sb = qio.tile([128, s], F32, tag="q")
                qT_sb = qio.tile([128, s], BF16, tag="qT")
                o_sb = qio.tile([128, s], F32, tag="o")
                kv_sb = qio.tile([128, 128], BF16, tag="kv")

                # loads: partition p holds rows 32p..32p+31 (16KB contiguous);
                # chunk j is the strided row-set {32p+j}.
                k3 = k[p].rearrange("(p n) d -> p n d", p=128)
                v3 = v[p].rearrange("(p n) d -> p n d", p=128)
                q3 = q[p].rearrange("(p n) d -> p n d", p=128)
                q_t3 = q_sb[:].rearrange("p (n d) -> p n d", d=128)
                # loads on the SP HWDGE ring (issuing from nc.scalar would
                # head-of-line block behind the qT copies on ACT, +15us)
                qtr = nch // 4
                if not last:
                    k_t3 = k_sb[:].rearrange("p (n d) -> p n d", d=128)
                    v_t3 = v_sb[:].rearrange("p (n d) -> p n d", d=128)
                    for h in range(4):
                        hs = ts(h, qtr)
                        nc.sync.dma_start(out=k_t3[:, hs], in_=k3[:, hs])
                        nc.sync.dma_start(out=v_t3[:, hs], in_=v3[:, hs])
                if p == pairs - 2:
                    # emit the LAST pair's k,v as f32->bf16 cast loads on the
                    # SWDGE ring (the only ring that casts in flight).  In
                    # that queue they sit after stores(p-1), so they issue
                    # while this pair computes and finish well before the
                    # last pair's q on the sync ring.  bf16 k,v also cuts
                    # the last pair's phase-A tail 4x (1 cycle/row).
                    kv3[0] = kv16p.tile([128, s], BF16, tag="k16", name="k16_sb")
                    kv3[1] = kv16p.tile([128, s], BF16, tag="v16", name="v16_sb")
                    kl3 = k[pairs - 1].rearrange("(p n) d -> p n d", p=128)
                    vl3 = v[pairs - 1].rearrange("(p n) d -> p n d", p=128)
                    k16t = kv3[0][:].rearrange("p (n d) -> p n d", d=128)
                    v16t = kv3[1][:].rearrange("p (n d) -> p n d", d=128)
                    for h in range(4):
                        hs = ts(h, qtr)
                        nc.gpsimd.dma_start(out=k16t[:, hs], in_=kl3[:, hs])
                        nc.gpsimd.dma_start(out=v16t[:, hs], in_=vl3[:, hs])
                # q in quarters; the last pair's final quarter is split into
                # two eighths so the very last load DMA gates only one
                # 4-chunk group of transposes/matmuls/stores.
                qspans = [(0, 8), (8, 8), (16, 8), (24, 8)]
                if last:
                    qspans = [(0, 8), (8, 8), (16, 8), (24, 4), (28, 4)]
                for st, ln in qspans:
                    hs = ds(st, ln)
                    nc.sync.dma_start(out=q_t3[:, hs], in_=q3[:, hs])
                if last:
                    # sacrificial final ring entry: the DGE services a ring's
                    # LAST descriptors at single-engine rate (~25GB/s, ~4-5us
                    # of crawl on the final ~100KB).  Re-reading two q chunks
                    # into a scratch tile nobody waits on moves that crawl
                    # onto bytes that gate nothing; the real final eighth
                    # gets full-rate service and the crawl overlaps the
                    # B-chain and stores that follow it.
                    scr = cpool.tile([128, 256], F32, name="tail_scratch")
                    scr3 = scr[:].rearrange("p (n d) -> p n d", d=128)
                    nc.sync.dma_start(out=scr3[:, ds(0, 2)], in_=q3[:, ds(0, 2)])

                # phase A: kv[d,e] accumulated over s-chunks, exact fp32;
                # the PSUM->SBUF copy rounds to bf16 for phase B.
                kv_ps = pskv.tile([128, 128], F32, tag="kv_ps")
                for n in range(nch):
                    nc.tensor.matmul(
                        kv_ps[:],
                        lhsT=k_sb[:, ts(n, 128)],
                        rhs=v_sb[:, ts(n, 128)],
                        start=(n == 0),
                        stop=(n == nch - 1),
                    )
                nc.vector.tensor_copy(kv_sb[:], kv_ps[:])

                o3 = out[p].rearrange("(p n) d -> p n d", p=128)
                o_t3 = o_sb[:].rearrange("p (n d) -> p n d", d=128)

                def emit_T(g):
                    qt_ps = psq.tile([128, gsz * 128], F32, tag="qt_ps")
                    for j in range(gsz):
                        n = g * gsz + j
                        nc.tensor.transpose(
                            qt_ps[:, ts(j, 128)], q_sb[:, ts(n, 128)], ident[:]
                        )
                    # ACT, not DVE: keeps the copy off DVE's in-order queue
                    # (which carries the v-adds); the copy also rounds the
                    # fp32 transpose result to bf16 for phase B.
                    nc.scalar.copy(qT_sb[:, ts(g, gsz * 128)], qt_ps[:])

                def emit_B(g):
                    o_ps = pso.tile([128, gsz * 128], F32, tag="o_ps")
                    for j in range(gsz):
                        n = g * gsz + j
                        nc.tensor.matmul(
                            o_ps[:, ts(j, 128)],
                            lhsT=qT_sb[:, ts(n, 128)],
                            rhs=kv_sb[:],
                            start=True,
                            stop=True,
                        )
                    nc.vector.tensor_add(
                        o_sb[:, ts(g, gsz * 128)],
                        o_ps[:],
                        v_sb[:, ts(g, gsz * 128)],
                    )

                emit_T(0)
                stored = 0
                # stores per 8 chunks during the bulk, per 4-chunk group for
                # the last pair's final half so the tail store is small; all
                # on the SWDGE ring (its sequencer is otherwise idle at the
                # tail -- the sync ring still issues the last q loads).
                for g in range(ngrp):
                    if g + 1 < ngrp:
                        emit_T(g + 1)
                    emit_B(g)
                    done = (g + 1) * gsz
                    fine = last and done > 24
                    if done % 8 == 0 or g == ngrp - 1 or fine:
                        hs = ds(stored, done - stored)
                        nc.gpsimd.dma_start(out=o3[:, hs], in_=o_t3[:, hs])
                        stored = done
    nc.finalize()
    return nc


def kernel(q, k, v, _trace=False):
    q = np.ascontiguousarray(np.asarray(q, dtype=np.float32)).reshape(B * H, S, D)
    k = np.ascontiguousarray(np.asarray(k, dtype=np.float32)).reshape(B * H, S, D)
    v = np.ascontiguousarray(np.asarray(v, dtype=np.float32)).reshape(B * H, S, D)

    nc = build_nc()
    in_maps = [
        {
            "q": q[i * PAIRS : (i + 1) * PAIRS],
            "k": k[i * PAIRS : (i + 1) * PAIRS],
            "v": v[i * PAIRS : (i + 1) * PAIRS],
        }
        for i in range(N_CORES)
    ]
    res = run_bass_kernel_spmd(nc, in_maps, core_ids=list(range(N_CORES)))
    full = np.concatenate([res.results[i]["out"] for i in range(N_CORES)], axis=0)
    out = full.reshape(B, H, S, D)
    if _trace:
        # repeat traced executes: the executable is compiled+cached after the
        # first run, so each NTFF profile context wraps only an execute.
        # Multiple samples filter out co-tenant HBM-contention noise.
        tres = [
            run_bass_kernel_spmd(
                nc,
                in_maps,
                core_ids=list(range(N_CORES)),
                trace=True,
                trace_cores=list(range(N_CORES)),
            )
            for _ in range(3)
        ]
        return out, tres
    return out
